# revision 1
# baseline (speedup 1.0000x reference)
"""Entropy-regularized attention (standard MHA fwd) on 8 trn2 cores.

Sharding: core c -> batch b=c//4, head-group g=c%4 (4 of 16 heads).
Each core computes q/k/v for its 256-wide head-group slice, transposed-
layout attention (scores^T = K^T-stationary matmuls, exp on ACT, AV with
v-stationary producing avT), then a row-split Wo partial product.
Host sums the 4 partials per batch and adds bo (the "all-reduce").

All matmuls run as float32r (1 cycle/row vs 4 for plain fp32).
"""

import sys

for _p in ("/opt/trn_rl_repo", "/root/.axon_site/_ro/trn_rl_repo"):
    if _p not in sys.path:
        sys.path.insert(0, _p)

import numpy as np

import concourse.bass as bass
import concourse.mybir as mybir
import concourse.tile as tile
from concourse import bacc

P = 128
S = 2048  # sequence length
D = 1024  # hidden
DG = 256  # per-core head-group width (4 heads x 64)
HD = 64
NHL = 4  # heads per core
KT_D = D // P  # 8 contraction tiles for projections
ST = S // P  # 16 sequence tiles
QG = 1024  # qi group size (PSUM budget: scores 2x2 banks + av 2 + rb 2)
NQG = S // QG

F32 = mybir.dt.float32
F32R = mybir.dt.float32r
BF16 = mybir.dt.bfloat16


def build_nc():
    nc = bacc.Bacc(None, target_bir_lowering=False)

    xT = nc.dram_tensor("xT", [D, S], F32R, kind="ExternalInput")
    wq = nc.dram_tensor("wq", [D, DG], F32R, kind="ExternalInput")
    wk = nc.dram_tensor("wk", [D, DG], F32R, kind="ExternalInput")
    wv = nc.dram_tensor("wv", [D, DG], F32R, kind="ExternalInput")
    wo = nc.dram_tensor("wo", [DG, D], F32R, kind="ExternalInput")
    bq = nc.dram_tensor("bq", [P, 2], F32, kind="ExternalInput")
    bk = nc.dram_tensor("bk", [P, 2], F32, kind="ExternalInput")
    bv = nc.dram_tensor("bv", [1, DG], F32R, kind="ExternalInput")
    out = nc.dram_tensor("out", [S, D], F32, kind="ExternalOutput")

    with tile.TileContext(nc) as tc:
        _body(tc, nc, xT, wq, wk, wv, wo, bq, bk, bv, out)

    # Pin Exp/Ln to the one table set holding both: strip them from the
    # competing sets (dict order and size preserved, so act_func_set_id
    # indices stay valid). Without this the table-load pass alternates
    # exp_and_others <-> natural_log per head (~17 x 1.3us + PE stalls).
    import concourse.bacc as _bacc_mod

    _orig_tables = _bacc_mod.get_activation_tables

    def _pinned_tables(arch):
        t = _orig_tables(arch)
        for name, fns in t.items():
            if name != "natural_log_exp_and_others":
                fns.discard(mybir.ActivationFunctionType.Exp)
                fns.discard(mybir.ActivationFunctionType.Ln)
        return t

    _bacc_mod.get_activation_tables = _pinned_tables
    try:
        nc.compile()
    finally:
        _bacc_mod.get_activation_tables = _orig_tables
    return nc


def _body(tc, nc, xT, wq, wk, wv, wo, bq, bk, bv, out):
    from contextlib import ExitStack

    with ExitStack() as ctx:
        ctx.enter_context(
            nc.allow_low_precision(
                reason="float32r/bf16 matmul inputs; accumulation is fp32 PSUM"
            )
        )
        persist = ctx.enter_context(tc.tile_pool(name="persist", bufs=1))
        expool = ctx.enter_context(tc.tile_pool(name="expool", bufs=3))
        npool = ctx.enter_context(tc.tile_pool(name="npool", bufs=2))
        opool = ctx.enter_context(tc.tile_pool(name="opool", bufs=3))
        # PSUM budget (8 banks): sc 2x[128,1024]=4, av 1x[128,1024]=2,
        # ops 2x[128,512]=2. qkv/v/rb/proj groups borrow sc/ops slots.
        ps_sc = ctx.enter_context(tc.tile_pool(name="ps_sc", bufs=2, space="PSUM"))
        ps_av = ctx.enter_context(tc.tile_pool(name="ps_av", bufs=1, space="PSUM"))
        ps_o = ctx.enter_context(tc.tile_pool(name="ps_o", bufs=2, space="PSUM"))

        qT_sb = persist.tile([P, 2, S], BF16)
        kT_sb = persist.tile([P, 2, S], BF16)
        v_sb = persist.tile([P, ST, NHL * 65], BF16)  # 65-striped: col 64 = ones
        avT = [
            persist.tile([P, 2, QG], F32R, tag=f"avT{g}", name=f"avT{g}")
            for g in range(NQG)
        ]
        wo_sb = persist.tile([P, 2, D], F32R)
        ones_row = persist.tile([1, P], F32R)
        xT_sb = persist.tile([P, KT_D, S], F32R)
        wq_sb = persist.tile([P, KT_D, DG], F32R, tag="wq")
        wk_sb = persist.tile([P, KT_D, DG], F32R, tag="wk")
        wv_sb = persist.tile([P, KT_D, DG], F32R, tag="wv")
        bq_sb = persist.tile([P, 2], F32, tag="bq")
        bk_sb = persist.tile([P, 2], F32, tag="bk")
        bv_sb = persist.tile([1, DG], F32R, tag="bv")

        # weights/biases first (small), then x^T in per-kt chunks so the
        # first projection matmuls start as soon as chunk 0 lands
        nc.sync.dma_start(bq_sb[:], bq[:])
        nc.sync.dma_start(bk_sb[:], bk[:])
        nc.sync.dma_start(bv_sb[:], bv[:])
        nc.sync.dma_start(wk_sb[:], wk.rearrange("(kt p) n -> p kt n", p=P))
        nc.sync.dma_start(wq_sb[:], wq.rearrange("(kt p) n -> p kt n", p=P))
        nc.sync.dma_start(wv_sb[:], wv.rearrange("(kt p) n -> p kt n", p=P))
        nc.sync.dma_start(wo_sb[:], wo.rearrange("(kt p) n -> p kt n", p=P))
        xTr = xT.rearrange("(kt p) s -> p kt s", p=P)
        for kt in range(KT_D):
            nc.sync.dma_start(xT_sb[:, kt], xTr[:, kt])

        # memset can't emit float32r; stage fp32 ones and copy-cast (rounds)
        ones_f32 = persist.tile([P, P], F32)
        nc.vector.memset(ones_f32[:], 1.0)
        nc.vector.tensor_copy(ones_row[:], ones_f32[0:1, :])
        nc.vector.tensor_copy(
            v_sb.rearrange("p st (h w) -> p st h w", w=65)[:, :, :, 64],
            ones_f32[:, 0:64].rearrange("p (st h) -> p st h", h=4),
        )

        # ---- projection building blocks (emitted on demand) ----
        def qk_group(wsb, bsb, dest, mt, nq):
            # one [128, 512] slice of qT/kT: 8-MM K-chain + bias eviction.
            # Borrows an sc-pool slot (free whenever ACT owns the pace).
            ps = ps_sc.tile([P, QG], F32, tag="sc", name="qkps")
            for kt in range(KT_D):
                nc.tensor.matmul(
                    ps[:, 0:512],
                    wsb[:, kt, mt * P : (mt + 1) * P],
                    xT_sb[:, kt, nq * 512 : (nq + 1) * 512],
                    start=(kt == 0),
                    stop=(kt == KT_D - 1),
                )
            nc.vector.tensor_scalar_add(
                dest[:, mt, nq * 512 : (nq + 1) * 512],
                ps[:, 0:512],
                bsb[:, mt : mt + 1],
            )

        def v_group(st):
            ps = ps_o.tile([P, 512], F32, tag="ops", name="vps")
            for kt in range(KT_D):
                nc.tensor.matmul(
                    ps[:, 0:DG],
                    xT_sb[:, kt, st * P : (st + 1) * P],
                    wv_sb[:, kt, :],
                    start=(kt == 0),
                    stop=False,
                )
            nc.tensor.matmul(
                ps[:, 0:DG],
                ones_row[0:1, 0:P],
                bv_sb[0:1, :],
                start=False,
                stop=True,
            )
            nc.vector.tensor_copy(
                v_sb.rearrange("p st (h w) -> p st h w", w=65)[:, st, :, 0:64],
                ps[:, 0:DG].rearrange("p (h w) -> p h w", w=64),
            )

        # mt0 projections + v up front (heads 0/1 + all AV need them)...
        for nq in range(S // 512):
            qk_group(wk_sb, bk_sb, kT_sb, 0, nq)
            qk_group(wq_sb, bq_sb, qT_sb, 0, nq)
        for st in range(ST):
            v_group(st)
        # ...mt1 groups deferred: sprinkled into the ACT-paced attention
        # loop of heads 0/1 to fill PE idle and keep HAM warm
        deferred = [
            (wsb, bsb, dest, 1, nq)
            for nq in range(S // 512)
            for (wsb, bsb, dest) in ((wk_sb, bk_sb, kT_sb), (wq_sb, bq_sb, qT_sb))
        ]

        def emit_norm(qg, h, av):
            # normalize: avT_norm = av[0:64] * (1 / av[64]) bcast over d.
            # 1/l via exp(-ln(l)) on ACT (same table set as the big exps);
            # DVE reciprocal is ~6.5us for [1,1024] and stalls the PE.
            mt, po = h // 2, (h % 2) * 64
            l_sb = npool.tile([1, QG], F32, tag="lsb")
            nc.scalar.activation(
                l_sb[:], av[64:65, :], mybir.ActivationFunctionType.Ln
            )
            r_row = npool.tile([1, QG], F32R, tag="rrow")
            nc.scalar.activation(
                r_row[:],
                l_sb[:],
                mybir.ActivationFunctionType.Exp,
                scale=-1.0,
            )
            u_sb = npool.tile([64, QG], F32R, tag="usb")
            nc.vector.tensor_copy(u_sb[:], av[0:64, :])
            for nq in range(QG // 512):
                rb = ps_o.tile([P, 512], F32, tag="ops", name="rb")
                nc.tensor.matmul(
                    rb[0:64, :],
                    ones_row[0:1, 0:64],
                    r_row[0:1, nq * 512 : (nq + 1) * 512],
                    start=True,
                    stop=True,
                )
                nc.vector.tensor_mul(
                    out=avT[qg][po : po + 64, mt, nq * 512 : (nq + 1) * 512],
                    in0=u_sb[:, nq * 512 : (nq + 1) * 512],
                    in1=rb[0:64, :],
                )

        pending_norm = None
        for qg in range(NQG):
            q0 = qg * QG
            for h in range(NHL):
                mt, po = h // 2, (h % 2) * 64
                av = ps_av.tile([P, QG], F32, tag="av")
                for kt in range(ST):
                    sc = ps_sc.tile([P, QG], F32, tag="sc")
                    for nq in range(QG // 512):
                        nc.tensor.matmul(
                            sc[:, nq * 512 : (nq + 1) * 512],
                            kT_sb[po : po + 64, mt, kt * P : (kt + 1) * P],
                            qT_sb[
                                po : po + 64,
                                mt,
                                q0 + nq * 512 : q0 + (nq + 1) * 512,
                            ],
                            start=True,
                            stop=True,
                        )
                    ex = expool.tile([P, QG], BF16, tag="ex")
                    nc.scalar.activation(
                        ex[:],
                        sc[:],
                        mybir.ActivationFunctionType.Exp,
                        scale=0.125,
                    )
                    # fill PE idle in this ACT-paced loop with deferred work:
                    # previous head's norm, then mt1 projection groups
                    if kt == 2 and pending_norm is not None:
                        pending_norm()
                        pending_norm = None
                    if kt in (4, 8, 12, 15) and deferred:
                        wsb, bsb, dest, dmt, dnq = deferred.pop(0)
                        qk_group(wsb, bsb, dest, dmt, dnq)
                    for nq in range(QG // 512):
                        nc.tensor.matmul(
                            av[0:65, nq * 512 : (nq + 1) * 512],
                            v_sb[:, kt, h * 65 : h * 65 + 65],
                            ex[:, nq * 512 : (nq + 1) * 512],
                            start=(kt == 0),
                            stop=(kt == ST - 1),
                        )
                pending_norm = (
                    lambda qg=qg, h=h, av=av: emit_norm(qg, h, av)
                )

            # flush the last head's norm before its qi-group's projection
            if pending_norm is not None:
                pending_norm()
                pending_norm = None

            # Phase D for this qi-group (st tiles qg*8 .. qg*8+8)
            for sti in range(QG // P):
                st = qg * (QG // P) + sti
                po_ps = ps_o.tile([P, 512], F32, tag="ops")
                po_ps2 = ps_o.tile([P, 512], F32, tag="ops")
                ot = opool.tile([P, D], F32, tag="ot")
                for nd, pp in ((0, po_ps), (1, po_ps2)):
                    for kt2 in range(2):
                        nc.tensor.matmul(
                            pp[:],
                            avT[qg][:, kt2, sti * P : (sti + 1) * P],
                            wo_sb[:, kt2, nd * 512 : (nd + 1) * 512],
                            start=(kt2 == 0),
                            stop=(kt2 == 1),
                        )
                    nc.vector.tensor_copy(
                        ot[:, nd * 512 : (nd + 1) * 512], pp[:]
                    )
                nc.sync.dma_start(out[st * P : (st + 1) * P, :], ot[:])


_NC_CACHE = None


def get_nc():
    global _NC_CACHE
    if _NC_CACHE is None:
        _NC_CACHE = build_nc()
    return _NC_CACHE


def make_in_maps(x, Wq, bq, Wk, bk, Wv, bv, Wo, bo):
    in_maps = []
    for c in range(8):
        b, g = c // 4, c % 4
        sl = slice(g * DG, (g + 1) * DG)
        in_maps.append(
            {
                "xT": np.ascontiguousarray(x[b].T),
                "wq": np.ascontiguousarray(Wq[:, sl]),
                "wk": np.ascontiguousarray(Wk[:, sl]),
                "wv": np.ascontiguousarray(Wv[:, sl]),
                "wo": np.ascontiguousarray(Wo[sl, :]),
                "bq": np.ascontiguousarray(bq[sl].reshape(2, P).T),
                "bk": np.ascontiguousarray(bk[sl].reshape(2, P).T),
                "bv": np.ascontiguousarray(bv[sl].reshape(1, DG)),
            }
        )
    return in_maps


def kernel(x, Wq, bq, Wk, bk, Wv, bv, Wo, bo, _run_kwargs=None):
    from concourse.bass_utils import run_bass_kernel_spmd

    x = np.asarray(x, dtype=np.float32)
    nc = get_nc()
    in_maps = make_in_maps(
        x,
        np.asarray(Wq, np.float32),
        np.asarray(bq, np.float32),
        np.asarray(Wk, np.float32),
        np.asarray(bk, np.float32),
        np.asarray(Wv, np.float32),
        np.asarray(bv, np.float32),
        np.asarray(Wo, np.float32),
        np.asarray(bo, np.float32),
    )
    res = run_bass_kernel_spmd(
        nc, in_maps, core_ids=list(range(8)), **(_run_kwargs or {})
    )
    bo = np.asarray(bo, np.float32)
    outp = np.empty((2, S, D), dtype=np.float32)
    for b in range(2):
        acc = res.results[4 * b]["out"].astype(np.float32)
        for g in range(1, 4):
            acc = acc + res.results[4 * b + g]["out"]
        outp[b] = acc + bo[None, :]
    kernel.last_result = res
    return outp



# revision 10
# speedup vs baseline: 1.3666x; 1.3666x over previous
"""Entropy-regularized attention (standard MHA fwd) on 8 trn2 cores.

Sharding: core c -> batch b=c//4, head-group g=c%4 (4 of 16 heads).
Each core computes q/k/v for its 256-wide head-group slice, transposed-
layout attention (scores^T = K^T-stationary matmuls, exp on ACT, AV with
v-stationary producing avT), then a row-split Wo partial product.
Host sums the 4 partials per batch and adds bo (the "all-reduce").

v2: ACT-paced software pipeline. The exp stream (128 x ACTIVATE[128,1024],
~1.11us each) is the theoretical floor; all projection / output-projection
/ normalization work is chopped into <=0.5us "filler" pieces emitted
between pipeline rounds so the PE runs them in exp shadows. bf16 matmul
inputs, fp16 output partials, DVE reciprocal (not ACT Ln/Exp) for the
softmax denominators.
"""

import sys

for _p in ("/opt/trn_rl_repo", "/root/.axon_site/_ro/trn_rl_repo"):
    if _p not in sys.path:
        sys.path.insert(0, _p)

import numpy as np

import concourse.bass as bass
import concourse.mybir as mybir
import concourse.tile as tile
from concourse import bacc

P = 128
S = 2048  # sequence length
D = 1024  # hidden
DG = 256  # per-core head-group width (4 heads x 64)
HD = 64
NHL = 4  # heads per core
KT_D = D // P  # 8 contraction tiles for projections
ST = S // P  # 16 sequence tiles
QG = 1024  # qi group size
NQG = S // QG

F32 = mybir.dt.float32
F32R = mybir.dt.float32r
F16 = mybir.dt.float16
BF16 = mybir.dt.bfloat16


def build_nc():
    nc = bacc.Bacc(None, target_bir_lowering=False)

    xT = nc.dram_tensor("xT", [D, S], BF16, kind="ExternalInput")
    wq = nc.dram_tensor("wq", [D, DG], BF16, kind="ExternalInput")
    wk = nc.dram_tensor("wk", [D, DG], BF16, kind="ExternalInput")
    wv = nc.dram_tensor("wv", [D, DG], BF16, kind="ExternalInput")
    wo = nc.dram_tensor("wo", [DG, D], BF16, kind="ExternalInput")
    bq = nc.dram_tensor("bq", [P, 2], F32, kind="ExternalInput")
    bk = nc.dram_tensor("bk", [P, 2], F32, kind="ExternalInput")
    bv = nc.dram_tensor("bv", [1, DG], BF16, kind="ExternalInput")
    out = nc.dram_tensor("out", [S, D], F16, kind="ExternalOutput")

    with tile.TileContext(nc) as tc:
        _body(tc, nc, xT, wq, wk, wv, wo, bq, bk, bv, out)

    nc.compile()
    return nc


def _body(tc, nc, xT, wq, wk, wv, wo, bq, bk, bv, out):
    from contextlib import ExitStack

    with ExitStack() as ctx:
        ctx.enter_context(
            nc.allow_low_precision(
                reason="bf16 matmul inputs; accumulation is fp32 PSUM"
            )
        )
        persist = ctx.enter_context(tc.tile_pool(name="persist", bufs=1))
        expool = ctx.enter_context(tc.tile_pool(name="expool", bufs=3))
        npool = ctx.enter_context(tc.tile_pool(name="npool", bufs=2))
        opool = ctx.enter_context(tc.tile_pool(name="opool", bufs=2))
        # PSUM budget (8 banks): sc 2x[128,1024]=4, av 1x[128,1024]=2,
        # fill 2x[128,512]=2.
        ps_sc = ctx.enter_context(tc.tile_pool(name="ps_sc", bufs=2, space="PSUM"))
        ps_av = ctx.enter_context(tc.tile_pool(name="ps_av", bufs=1, space="PSUM"))
        ps_fl = ctx.enter_context(tc.tile_pool(name="ps_fl", bufs=2, space="PSUM"))

        qT_sb = persist.tile([P, 2, S], BF16)
        kT_sb = persist.tile([P, 2, S], BF16)
        v_sb = persist.tile([P, ST, NHL * 65], BF16)  # 65-striped: col 64 = ones
        avT = [
            persist.tile([P, 2, QG], BF16, tag=f"avT{g}", name=f"avT{g}")
            for g in range(NQG)
        ]
        wo_sb = persist.tile([P, 2, D], BF16)
        ones_bf = persist.tile([1, P], BF16)
        ones_r = persist.tile([1, P], F32R)
        xT_sb = persist.tile([P, KT_D, S], BF16)
        wq_sb = persist.tile([P, KT_D, DG], BF16, tag="wq")
        wk_sb = persist.tile([P, KT_D, DG], BF16, tag="wk")
        wv_sb = persist.tile([P, KT_D, DG], BF16, tag="wv")
        bq_sb = persist.tile([P, 2], F32, tag="bq")
        bk_sb = persist.tile([P, 2], F32, tag="bk")
        bv_sb = persist.tile([1, DG], BF16, tag="bv")

        # DMA order matters: wk/wq + the first two xT seq-quarters gate the
        # first exp; wv gates V (needed ~1 round later); wo only at phase D.
        nc.sync.dma_start(bq_sb[:], bq[:])
        nc.sync.dma_start(bk_sb[:], bk[:])
        nc.sync.dma_start(wk_sb[:], wk.rearrange("(kt p) n -> p kt n", p=P))
        nc.sync.dma_start(wq_sb[:], wq.rearrange("(kt p) n -> p kt n", p=P))
        xTr = xT.rearrange("(kt p) s -> p kt s", p=P)
        for sq in range(4):
            sl = slice(sq * 512, (sq + 1) * 512)
            for kt in range(KT_D):
                nc.sync.dma_start(xT_sb[:, kt, sl], xTr[:, kt, sl])
            if sq == 0:
                nc.sync.dma_start(bv_sb[:], bv[:])
                nc.sync.dma_start(
                    wv_sb[:], wv.rearrange("(kt p) n -> p kt n", p=P)
                )
        nc.sync.dma_start(wo_sb[:], wo.rearrange("(kt p) n -> p kt n", p=P))

        # constants: bf16 ones row, f32r ones row (for the rinv broadcast
        # matmul), and the ones stripe in v (softmax denominator column)
        ones_f32 = persist.tile([P, P], F32)
        nc.vector.memset(ones_f32[:], 1.0)
        nc.vector.tensor_copy(ones_bf[:], ones_f32[0:1, :])
        nc.vector.tensor_copy(ones_r[:], ones_f32[0:1, :])
        nc.vector.tensor_copy(
            v_sb.rearrange("p st (h w) -> p st h w", w=65)[:, :, :, 64],
            ones_f32[:, 0:64].rearrange("p (st h) -> p st h", h=4),
        )

        # ---- filler generators (emitted in <=~0.5us PE pieces) ----
        # `done` tracks which producer groups have been fully EMITTED.
        # Tile tracks deps in emission order, so a consumer emitted before
        # its producer sees a stale tile (WAR instead of RAW) — guards
        # below pump the queue until the producer is out.
        done = set()

        def gen_qk(kind, wsb, bsb, dest, mt, nq):
            # one [128, 512] slice of qT/kT: 8-MM K-chain + bias eviction
            ps = ps_fl.tile([P, 512], F32, tag="fill", name="qkps")
            for kt in range(KT_D):
                nc.tensor.matmul(
                    ps[:],
                    wsb[:, kt, mt * P : (mt + 1) * P],
                    xT_sb[:, kt, nq * 512 : (nq + 1) * 512],
                    start=(kt == 0),
                    stop=(kt == KT_D - 1),
                )
                if kt in (2, 5):
                    yield
            nc.vector.tensor_scalar_add(
                dest[:, mt, nq * 512 : (nq + 1) * 512],
                ps[:],
                bsb[:, mt : mt + 1],
            )
            done.add((kind, mt, nq))
            yield

        def gen_v(st):
            ps = ps_fl.tile([P, 512], F32, tag="fill", name="vps")
            for kt in range(KT_D):
                nc.tensor.matmul(
                    ps[:, 0:DG],
                    xT_sb[:, kt, st * P : (st + 1) * P],
                    wv_sb[:, kt, :],
                    start=(kt == 0),
                    stop=False,
                )
                if kt == 4:
                    yield
            nc.tensor.matmul(
                ps[:, 0:DG],
                ones_bf[0:1, 0:P],
                bv_sb[0:1, :],
                start=False,
                stop=True,
            )
            nc.vector.tensor_copy(
                v_sb.rearrange("p st (h w) -> p st h w", w=65)[:, st, :, 0:64],
                ps[:, 0:DG].rearrange("p (h w) -> p h w", w=64),
            )
            done.add(("v", st))
            yield

        def gen_norm(qg, h, ue, rinv):
            # avT[qg][head] = ue[0:64] * rinv  (bcast over d via ones matmul)
            mt, po = h // 2, (h % 2) * 64
            for nq in range(QG // 512):
                rb = ps_fl.tile([P, 512], F32, tag="fill", name="rb")
                nc.tensor.matmul(
                    rb[0:64, :],
                    ones_r[0:1, 0:64],
                    rinv[0:1, nq * 512 : (nq + 1) * 512],
                    start=True,
                    stop=True,
                )
                nc.vector.tensor_mul(
                    out=avT[qg][po : po + 64, mt, nq * 512 : (nq + 1) * 512],
                    in0=ue[0:64, nq * 512 : (nq + 1) * 512],
                    in1=rb[0:64, :],
                )
                yield

        def gen_phaseD(qg, sti):
            st = qg * (QG // P) + sti
            ot = opool.tile([P, D], F16, tag="ot")
            for nd in range(2):
                pp = ps_fl.tile([P, 512], F32, tag="fill", name="pp")
                for kt2 in range(2):
                    nc.tensor.matmul(
                        pp[:],
                        avT[qg][:, kt2, sti * P : (sti + 1) * P],
                        wo_sb[:, kt2, nd * 512 : (nd + 1) * 512],
                        start=(kt2 == 0),
                        stop=(kt2 == 1),
                    )
                nc.vector.tensor_copy(ot[:, nd * 512 : (nd + 1) * 512], pp[:])
                yield
            nc.sync.dma_start(out[st * P : (st + 1) * P, :], ot[:])
            yield

        # ---- filler queue ----
        from collections import deque

        fill_q = deque()
        cur_gen = [None]

        def pump(n):
            for _ in range(n):
                while True:
                    if cur_gen[0] is None:
                        if not fill_q:
                            return
                        cur_gen[0] = fill_q.popleft()
                    try:
                        next(cur_gen[0])
                        break
                    except StopIteration:
                        cur_gen[0] = None

        def pump_until(key):
            while key not in done:
                assert cur_gen[0] is not None or fill_q, f"missing {key}"
                pump(1)

        # pre-attention (emitted directly, highest priority): the minimal
        # set gating exp(qg0,h0,kt=0): K-mt0-nq0, Q-mt0-nq0/1, then V st0/1
        for g in (
            gen_qk("k", wk_sb, bk_sb, kT_sb, 0, 0),
            gen_qk("q", wq_sb, bq_sb, qT_sb, 0, 0),
            gen_qk("q", wq_sb, bq_sb, qT_sb, 0, 1),
            gen_v(0),
            gen_v(1),
        ):
            for _ in g:
                pass

        # remaining work ordered by first-use round:
        # K-nq j gates sc(kt=4j); V st gates av(kt=st); mt1 K/Q gate h2 (r32)
        fill_q.extend(
            [
                gen_v(2),
                gen_qk("k", wk_sb, bk_sb, kT_sb, 0, 1),
                gen_v(3),
                gen_v(4),
                gen_v(5),
                gen_qk("k", wk_sb, bk_sb, kT_sb, 0, 2),
                gen_v(6),
                gen_v(7),
                gen_v(8),
                gen_qk("k", wk_sb, bk_sb, kT_sb, 0, 3),
                gen_v(9),
                gen_v(10),
                gen_v(11),
                gen_v(12),
                gen_v(13),
                gen_v(14),
                gen_v(15),
                gen_qk("k", wk_sb, bk_sb, kT_sb, 1, 0),
                gen_qk("q", wq_sb, bq_sb, qT_sb, 1, 0),
                gen_qk("q", wq_sb, bq_sb, qT_sb, 1, 1),
                gen_qk("k", wk_sb, bk_sb, kT_sb, 1, 1),
                gen_qk("k", wk_sb, bk_sb, kT_sb, 1, 2),
                gen_qk("k", wk_sb, bk_sb, kT_sb, 1, 3),
                gen_qk("q", wq_sb, bq_sb, qT_sb, 0, 2),
                gen_qk("q", wq_sb, bq_sb, qT_sb, 0, 3),
                gen_qk("q", wq_sb, bq_sb, qT_sb, 1, 2),
                gen_qk("q", wq_sb, bq_sb, qT_sb, 1, 3),
            ]
        )
        done.update({("k", 0, 0), ("q", 0, 0), ("q", 0, 1), ("v", 0), ("v", 1)})

        # ---- ACT-paced attention rounds ----
        blocks = [(qg, h) for qg in range(NQG) for h in range(NHL)]
        for bi, (qg, h) in enumerate(blocks):
            mt, po = h // 2, (h % 2) * 64
            q0 = qg * QG
            av = ps_av.tile([P, QG], F32, tag="av")
            pump_until(("q", mt, 2 * qg))
            pump_until(("q", mt, 2 * qg + 1))
            prev = None  # (ex, kt) awaiting its AV matmuls
            for kt in range(ST):
                pump_until(("k", mt, kt // 4))
                sc = ps_sc.tile([P, QG], F32, tag="sc")
                for nq in range(QG // 512):
                    nc.tensor.matmul(
                        sc[:, nq * 512 : (nq + 1) * 512],
                        kT_sb[po : po + 64, mt, kt * P : (kt + 1) * P],
                        qT_sb[
                            po : po + 64,
                            mt,
                            q0 + nq * 512 : q0 + (nq + 1) * 512,
                        ],
                        start=True,
                        stop=True,
                    )
                ex = expool.tile([P, QG], BF16, tag="ex")
                nc.scalar.activation(
                    ex[:],
                    sc[:],
                    mybir.ActivationFunctionType.Exp,
                    scale=0.125,
                )
                if prev is not None:
                    pex, pkt = prev
                    pump_until(("v", pkt))
                    for nq in range(QG // 512):
                        nc.tensor.matmul(
                            av[0:65, nq * 512 : (nq + 1) * 512],
                            v_sb[:, pkt, h * 65 : h * 65 + 65],
                            pex[:, nq * 512 : (nq + 1) * 512],
                            start=(pkt == 0),
                            stop=(pkt == ST - 1),
                        )
                prev = (ex, kt)
                pump(2 if bi == 0 else 1)
            pex, pkt = prev
            pump_until(("v", pkt))
            for nq in range(QG // 512):
                nc.tensor.matmul(
                    av[0:65, nq * 512 : (nq + 1) * 512],
                    v_sb[:, pkt, h * 65 : h * 65 + 65],
                    pex[:, nq * 512 : (nq + 1) * 512],
                    start=False,
                    stop=True,
                )

            # evict av -> SBUF (frees the av bank), 1/l on DVE, then the
            # normalize runs as filler during the next block
            ue = npool.tile([65, QG], F32, tag="ue")
            nc.vector.tensor_copy(ue[:], av[0:65, :])
            # reciprocal_approx_fast misbehaves at partition offset 64:
            # shift l down to partition 0 first
            l0 = npool.tile([1, QG], F32, tag="l0")
            nc.vector.tensor_copy(l0[:], ue[64:65, :])
            rinv = npool.tile([1, QG], F32, tag="rinv")
            nc.vector.reciprocal_approx_fast(rinv[:], l0[:])
            rinv_r = npool.tile([1, QG], F32R, tag="rinvr")
            nc.vector.tensor_copy(rinv_r[:], rinv[:])
            fill_q.append(gen_norm(qg, h, ue, rinv_r))

            if (qg, h) == (0, NHL - 1):
                # qg0 output projection: runs as filler through qg1
                for sti in range(QG // P):
                    fill_q.append(gen_phaseD(0, sti))

        # tail: drain remaining filler, then qg1 output projection
        pump(1 << 20)
        for sti in range(QG // P):
            for _ in gen_phaseD(1, sti):
                pass


_NC_CACHE = None


def get_nc():
    global _NC_CACHE
    if _NC_CACHE is None:
        _NC_CACHE = build_nc()
    return _NC_CACHE


def make_in_maps(x, Wq, bq, Wk, bk, Wv, bv, Wo, bo):
    import ml_dtypes

    bf16 = ml_dtypes.bfloat16
    in_maps = []
    for c in range(8):
        b, g = c // 4, c % 4
        sl = slice(g * DG, (g + 1) * DG)
        in_maps.append(
            {
                "xT": np.ascontiguousarray(x[b].T.astype(bf16)),
                "wq": np.ascontiguousarray(Wq[:, sl].astype(bf16)),
                "wk": np.ascontiguousarray(Wk[:, sl].astype(bf16)),
                "wv": np.ascontiguousarray(Wv[:, sl].astype(bf16)),
                "wo": np.ascontiguousarray(Wo[sl, :].astype(bf16)),
                "bq": np.ascontiguousarray(bq[sl].reshape(2, P).T),
                "bk": np.ascontiguousarray(bk[sl].reshape(2, P).T),
                "bv": np.ascontiguousarray(bv[sl].reshape(1, DG).astype(bf16)),
            }
        )
    return in_maps


def kernel(x, Wq, bq, Wk, bk, Wv, bv, Wo, bo, _run_kwargs=None):
    from concourse.bass_utils import run_bass_kernel_spmd

    x = np.asarray(x, dtype=np.float32)
    nc = get_nc()
    in_maps = make_in_maps(
        x,
        np.asarray(Wq, np.float32),
        np.asarray(bq, np.float32),
        np.asarray(Wk, np.float32),
        np.asarray(bk, np.float32),
        np.asarray(Wv, np.float32),
        np.asarray(bv, np.float32),
        np.asarray(Wo, np.float32),
        np.asarray(bo, np.float32),
    )
    res = run_bass_kernel_spmd(
        nc, in_maps, core_ids=list(range(8)), **(_run_kwargs or {})
    )
    bo = np.asarray(bo, np.float32)
    outp = np.empty((2, S, D), dtype=np.float32)
    for b in range(2):
        acc = res.results[4 * b]["out"].astype(np.float32)
        for g in range(1, 4):
            acc = acc + res.results[4 * b + g]["out"].astype(np.float32)
        outp[b] = acc + bo[None, :]
    kernel.last_result = res
    return outp
